# revision 1
# baseline (speedup 1.0000x reference)
"""DropSphereNd Trainium2 kernel.

Full computation (per sample n, channels c):
    activ = embeds @ table                      # [n, c]
    t     = 17th-smallest(activ, axis=1)        # [n, 1]
    out   = x * (activ >= t) * c/(c-16)

Sharding: data-parallel over batch n across 8 cores (x/embeds sharded,
table replicated).  Per core: x shard [8, 256, 56, 56] viewed as
[2048, 3136]; the mask is computed on-device (tiny matmul + iterative
min-extraction) and applied as a per-partition scalar multiply while
streaming x through SBUF.

Raw bass (no Tile): the pinned walrus codegen allows only ONE sync-wait
per compute instruction, so all cross-engine deps use standalone
wait_ge sequencer commands.

Engine plan:
  SP  (nc.sync)   - input DMAs (table, embT, ident, x tiles)
  ACT (nc.scalar) - output DMAs
  PE  (nc.tensor) - projection matmul + 2 mask-transpose matmuls
  DVE (nc.vector) - threshold search, mask build, streaming multiplies
"""

import sys

if "/opt/trn_rl_repo" not in sys.path:
    sys.path.insert(0, "/opt/trn_rl_repo")

from contextlib import ExitStack

import numpy as np

import concourse.bass as bass
from concourse import mybir
from concourse.bass_utils import run_bass_kernel_spmd

N, C, H, W = 64, 256, 56, 56
HW = H * W  # 3136
E = 16
NCORES = 8
NLOC = N // NCORES  # 8 samples per core
INDEX = 16  # ceil(C ** 0.5)
SCALE = float(C) / (C - INDEX)
F32 = mybir.dt.float32
BIG = 1.0e30
BUFS = 6  # x-tile ring slots (25 KB/partition each)

_NC_CACHE = {}


def _build_nc() -> bass.Bass:
    # detect_race_conditions only affects the interpreter: its raw-bass model
    # has no same-engine program-order edges, so every chained DVE op would be
    # flagged.  Cross-engine ordering is handled by the explicit sems below.
    nc = bass.Bass(detect_race_conditions=False)
    x = nc.dram_tensor("x", [NLOC * C, HW], F32, kind="ExternalInput")
    emb = nc.dram_tensor("embeds", [NLOC, E], F32, kind="ExternalInput")
    tab = nc.dram_tensor("table", [E, C], F32, kind="ExternalInput")
    out = nc.dram_tensor("out", [NLOC * C, HW], F32, kind="ExternalOutput")
    ident_d = nc.inline_tensor(np.eye(NLOC, dtype=np.float32), name="ident8")

    # row r = t*256 + 2*p + parity  ->  sample t, channel c = 2*p + parity
    x_t = x[:, :].rearrange("(t p two) f -> t p two f", p=128, two=2)
    o_t = out[:, :].rearrange("(t p two) f -> t p two f", p=128, two=2)

    with ExitStack() as ctx:
        sb = lambda name, shape: ctx.enter_context(nc.sbuf_tensor(name, shape, F32))
        ps = lambda name, shape: ctx.enter_context(nc.psum_tensor(name, shape, F32))

        tab_s = sb("tab_s", [E, C])
        embT = sb("embT", [E, NLOC])
        ident = sb("ident", [NLOC, NLOC])
        v = sb("v", [NLOC, C])
        v2 = sb("v2", [NLOC, C])
        mx = sb("mx", [NLOC, 8])
        m_even = sb("m_even", [NLOC, C // 2])
        m_odd = sb("m_odd", [NLOC, C // 2])
        mA = sb("mA", [C // 2, NLOC])
        mB = sb("mB", [C // 2, NLOC])
        xbuf = [sb(f"xbuf{i}", [128, 2, HW]) for i in range(BUFS)]

        activ_p = ps("activ_p", [NLOC, C])
        mA_p = ps("mA_p", [C // 2, NLOC])
        mB_p = ps("mB_p", [C // 2, NLOC])

        ld = ctx.enter_context(nc.semaphore("ld"))
        fz = ctx.enter_context(nc.semaphore("fz"))
        dv = ctx.enter_context(nc.semaphore("dv"))
        pe = ctx.enter_context(nc.semaphore("pe"))
        # per-ring-slot DMA sems: same-sem increments are serialized by the
        # slot lifecycle, so wait values are unambiguous (race-detector clean)
        xs = [ctx.enter_context(nc.semaphore(f"xs{i}")) for i in range(BUFS)]
        ss = [ctx.enter_context(nc.semaphore(f"ss{i}")) for i in range(BUFS)]

        block = ctx.enter_context(nc.Block())

        # x loads get the SP ring to themselves: the first x descriptor
        # generates immediately instead of queueing behind the smalls
        # (the transposed embeds load alone is 128 single-element
        # descriptors).  Smalls ride the ACT ring, idle until stores begin.
        @block.sync
        def _(sync):
            for t in range(NLOC):
                if t >= BUFS:
                    # slot free once the store of tile t-BUFS has drained
                    sync.wait_ge(ss[t % BUFS], 16 * (t // BUFS))
                sync.dma_start(out=xbuf[t % BUFS][:, :, :], in_=x_t[t]).then_inc(
                    xs[t % BUFS], 16
                )

        @block.tensor
        def _(tensor):
            tensor.wait_ge(ld, 48)  # tab_s + embT (+ident) resident
            tensor.matmul(
                activ_p[:, :], embT[:, :], tab_s[:, :], start=True, stop=True
            ).then_inc(pe, 1)
            tensor.wait_ge(dv, 2)  # m_even + m_odd built
            tensor.matmul(
                mA_p[:, :], m_even[:, :], ident[:, :], start=True, stop=True
            ).then_inc(pe, 1)
            tensor.matmul(
                mB_p[:, :], m_odd[:, :], ident[:, :], start=True, stop=True
            ).then_inc(pe, 1)

        # The 16 smallest of activ == the 16 largest of v = -activ.  DVE's
        # max (top-8 per partition) + match_replace (zap those 8) drop them
        # in two rounds; surviving lanes keep their value, zapped lanes hold
        # MINV, so the mask is one compare against an immediate.  No
        # data-dependent scalar operands anywhere: TensorScalarPtr fetches
        # its scalar at sequencer dispatch (ahead of the DVE pipe), so only
        # mA/mB -- real pointer operands of the streaming muls -- need a
        # sem fence.
        MINV = -1.0e30

        @block.vector
        def _(vector):
            vector.wait_ge(pe, 1)
            vector.tensor_scalar_mul(v[:, :], activ_p[:, :], -1.0)
            # match_replace prefetches its 8-value table at dispatch, ahead
            # of the DVE pipe -- fence each max before consuming it
            vector.max(mx[:, :], v[:, :]).then_inc(fz, 1)
            vector.wait_ge(fz, 1)
            vector.match_replace(
                out=v2[:, :], in_to_replace=mx[:, :], in_values=v[:, :],
                imm_value=MINV,
            )
            vector.max(mx[:, :], v2[:, :]).then_inc(fz, 1)
            vector.wait_ge(fz, 2)
            vector.match_replace(
                out=v2[:, :], in_to_replace=mx[:, :], in_values=v2[:, :],
                imm_value=MINV,
            )
            # keep[c] <=> v2[c] != MINV ; mask = keep * SCALE, channel-parity
            # split (immediate compare: real values are > MINV/2)
            v_pair = v2[:, :].rearrange("n (j two) -> n j two", two=2)
            for parity, m8 in ((0, m_even), (1, m_odd)):
                vector.tensor_scalar(
                    out=m8[:, :],
                    in0=v_pair[:, :, parity],
                    scalar1=MINV / 2,
                    scalar2=SCALE,
                    op0=mybir.AluOpType.is_ge,
                    op1=mybir.AluOpType.mult,
                ).then_inc(dv, 1)
            vector.wait_ge(pe, 3)
            vector.tensor_copy(mA[:, :], mA_p[:, :])
            vector.tensor_copy(mB[:, :], mB_p[:, :]).then_inc(dv, 1)
            vector.wait_ge(dv, 3)  # mA/mB committed before mul ptr-fetches
            for t in range(NLOC):
                vector.wait_ge(xs[t % BUFS], 16 * (t // BUFS + 1))
                xb = xbuf[t % BUFS]
                vector.tensor_scalar_mul(
                    xb[:, 0, :], xb[:, 0, :], mA[:, t : t + 1]
                )
                vector.tensor_scalar_mul(
                    xb[:, 1, :], xb[:, 1, :], mB[:, t : t + 1]
                ).then_inc(dv, 1)

        DV_BASE = 3  # dv value once masks + mA/mB copies are done

        @block.scalar
        def _(scalar):
            scalar.dma_start(out=tab_s[:, :], in_=tab[:, :]).then_inc(ld, 16)
            with nc.allow_non_contiguous_dma(reason="8x16 transposed load, 512B"):
                scalar.dma_start(
                    out=embT[:, :], in_=emb[:, :].rearrange("n e -> e n")
                ).then_inc(ld, 16)
            scalar.dma_start(out=ident[:, :], in_=ident_d[:, :]).then_inc(ld, 16)
            for t in range(NLOC):
                scalar.wait_ge(dv, DV_BASE + (t + 1))  # both muls of tile t done
                scalar.dma_start(out=o_t[t], in_=xbuf[t % BUFS][:, :, :]).then_inc(
                    ss[t % BUFS], 16
                )

    return nc


def _get_nc() -> bass.Bass:
    if "nc" not in _NC_CACHE:
        _NC_CACHE["nc"] = _build_nc()
    return _NC_CACHE["nc"]


def _in_maps(x, embeds, table):
    x = np.ascontiguousarray(np.asarray(x, dtype=np.float32))
    embeds = np.ascontiguousarray(np.asarray(embeds, dtype=np.float32))
    table = np.ascontiguousarray(np.asarray(table, dtype=np.float32))
    maps = []
    for i in range(NCORES):
        maps.append(
            {
                "x": x[i * NLOC : (i + 1) * NLOC].reshape(NLOC * C, HW),
                "embeds": embeds[i * NLOC : (i + 1) * NLOC],
                "table": table,
            }
        )
    return maps


def kernel(x, embeds, table):
    nc = _get_nc()
    res = run_bass_kernel_spmd(nc, _in_maps(x, embeds, table), list(range(NCORES)))
    shards = [
        np.asarray(res.results[i]["out"]).reshape(NLOC, C, H, W)
        for i in range(NCORES)
    ]
    return np.concatenate(shards, axis=0)


def kernel_profiled(x, embeds, table, **trace_kwargs):
    """Same as kernel() but with NTFF tracing; returns (output, BassKernelResults)."""
    nc = _get_nc()
    res = run_bass_kernel_spmd(
        nc, _in_maps(x, embeds, table), list(range(NCORES)), trace=True, **trace_kwargs
    )
    shards = [
        np.asarray(res.results[i]["out"]).reshape(NLOC, C, H, W)
        for i in range(NCORES)
    ]
    return np.concatenate(shards, axis=0), res

